# revision 18
# baseline (speedup 1.0000x reference)
"""Trainium2 Bass kernel for nn_AttentionModule (GNN attention pooling).

Math (reference):
    a_w = a_alpha[:,0] @ W_alpha ; b_w = b_alpha[:,0] @ W_alpha
    alpha_j = exp(a_w @ X[0] + X_j @ b_w)
    out = ((alpha @ X) / jnp.sum(alpha)) @ W_sum

Since the output is a ratio, the constant factor exp(a_w @ X[0]) cancels
exactly, so each device only needs one streaming pass over its shard of X:
    t_j = X_j . b_w ; e_j = exp(t_j)
    num = sum_j e_j * X_j   (D floats)   den = sum_j e_j   (1 float)
Host: reduce the 8 partials, divide, project through W_sum (tiny).

Sharding: X row-wise across 8 cores. Rows landing in zero-filled slots
contribute exp(0)=1 to den (subtracted exactly on the host) and 0 to num.

Datapath: X streams from HBM as f32 (full memory traffic), cast to bf16
during the DMA (SWDGE inline cast). Measured: SDMA engine 15 runs ~25%
slower than the rest on SWDGE traffic (descriptor-ring port contention)
and partition p maps to engine p%16, so uniform tiles leave every tile's
completion semaphore trailing engine 15 by up to 8 us. Fix: on the big
steady tiles, partitions p%16==15 carry S<R rows (S/R ~ 0.70), delivered
by single-strided-partition-dim DMAs (multi-dim partition APs miscompile
on the SWDGE cast path; a 5-way stride decomposition is exact). Edge
tiles are small and stay uniform (1 DMA each) since Q7 descriptor
generation costs ~0.63 us per dma_start. Every tile has a unique SBUF
buffer (whole bf16 shard ~6.4 MB) so the stream never stalls on compute.

num accumulates via 4-row batched matmuls (psum[4,512] +=
ev[:,4g:4g+4].T @ X[:, 4 blocks]); the host extracts the block-diagonal.
All accumulations (t dot products, den, PSUM num) stay in f32.
"""

import numpy as np

D = 128
NCORES = 8
N = 200000
# tiles: (R rows/partition, S rows on slow partitions; S == R -> uniform)
TILES = [(8, 8), (12, 12), (28, 20), (28, 20), (28, 20), (28, 20),
         (28, 20), (28, 20), (8, 8), (4, 4)]
T = len(TILES)
R_MAX = max(r for r, _ in TILES)
CAP = sum(120 * r + 8 * s for r, s in TILES)
ZSLOTS = sum(8 * (r - s) for r, s in TILES)
EXP0 = NCORES * ZSLOTS + (NCORES * CAP - N)  # total exp(0) contributions

_nc_cache = None
LAST_RESULTS = None


def _build():
    import concourse.bacc as bacc
    import concourse.bass as bass
    import concourse.mybir as mybir
    import concourse.tile as tile

    f32 = mybir.dt.float32
    bf16 = mybir.dt.bfloat16
    nc = bacc.Bacc("TRN2", target_bir_lowering=False, debug=False)

    NMM = sum(r // 4 for r, _ in TILES)   # 4-row batched matmuls

    x = nc.dram_tensor("x", [CAP, D], f32, kind="ExternalInput")
    bw = nc.dram_tensor("bw", [128, D], bf16, kind="ExternalInput")
    zt = nc.dram_tensor("zt", [8, R_MAX * D], bf16, kind="ExternalInput")
    out_num = nc.dram_tensor("out_num", [4, 512], f32, kind="ExternalOutput")
    out_den = nc.dram_tensor("out_den", [128, 1], f32, kind="ExternalOutput")

    with tile.TileContext(nc, pool_alloc_mode="queue") as tc:
        with (
            tc.tile_pool(name="xb", bufs=1) as xbpool,
            tc.tile_pool(name="pr", bufs=2) as prpool,
            tc.tile_pool(name="hv", bufs=2) as hvpool,
            tc.tile_pool(name="sm", bufs=3) as spool,
            tc.tile_pool(name="acc", bufs=1) as accpool,
            tc.tile_pool(name="ps", bufs=1, space=bass.MemorySpace.PSUM) as pspool,
            tc.tile_pool(name="pw", bufs=1, space=bass.MemorySpace.PSUM) as pwpool,
        ):
            bsmall = accpool.tile([128, D], bf16)
            nc.sync.dma_start(bsmall[:], bw[:, :])
            # replicate b_w R_MAX times along the free dim (one-time)
            bwt = accpool.tile([128, R_MAX * D], bf16)
            nc.vector.tensor_copy(
                bwt[:].rearrange("p (r d) -> p r d", r=R_MAX),
                bsmall[:].rearrange("p (u d) -> p u d", u=1).broadcast_to(
                    [128, R_MAX, D]
                ),
            )

            den_all = accpool.tile([128, T], f32)
            num_ps = pspool.tile([4, 512], f32, name="num_ps", tag="ps")

            # PE warm-up: ~5 us of dep-free junk matmuls during the DMA fill
            # flips the HAM clock gate to 2.4 GHz; the real per-tile bursts
            # (~1.5 us warm, ~2.9 us apart) then keep it warm, halving every
            # matmul and the end-of-stream flush. Own PSUM pool: sharing a
            # bank with num_ps serializes the real accumulation group.
            warm_ps = pwpool.tile([4, 512], f32, name="warm_ps", tag="warm")
            for w in range(12):
                nc.tensor.matmul(
                    warm_ps[:], bwt[:, 0:4], bwt[:, 0:512],
                    start=True, stop=True,
                )

            # unique bf16 buffer per tile + one-time zero fill of the slow
            # partitions' tail slots on ragged tiles (rows S..R never
            # DMA-written)
            xts = []
            for t, (R, S) in enumerate(TILES):
                xt = xbpool.tile([128, R * D], bf16, name=f"xt{t}", tag=f"xt{t}")
                if S < R:
                    nc.sync.dma_start(
                        xt[:].rearrange("(g j) f -> g j f", j=16)[
                            :, 15, S * D:R * D],
                        zt.ap()[:, 0:(R - S) * D],
                    )
                xts.append(xt)

            with nc.allow_low_precision("t stats kept in bf16; exp reads them"):
                row0 = 0
                i = 0
                for t, (R, S) in enumerate(TILES):
                    xt = xts[t]
                    if S == R:
                        # uniform tile: single SWDGE cast DMA, all partitions
                        nc.gpsimd.dma_start(
                            xt[:],
                            x.ap()[row0:row0 + 128 * R, :].rearrange(
                                "(p r) d -> p (r d)", p=128
                            ).opt(),
                        )
                        row0 += 128 * R
                    else:
                        # ragged tile: five single-strided-partition DMAs;
                        # p%16==15 (engine 15) carries S < R rows
                        for start, step, n, rr in [
                            (0, 2, 64, R), (1, 4, 32, R), (3, 8, 16, R),
                            (7, 16, 8, R), (15, 16, 8, S),
                        ]:
                            nc.gpsimd.dma_start(
                                xt[:].rearrange(
                                    "(g j) f -> g j f", j=step
                                )[:, start, 0:rr * D],
                                x.ap()[row0:row0 + n * rr, :].rearrange(
                                    "(g r) d -> g (r d)", g=n
                                ),
                            )
                            row0 += n * rr
                    xs = xt[:, 0:R * D]

                    # t_j = X_j . b_w : multiply at DVE 2x, shrink with
                    # 2x halving adds, then the 1x-capped reduce (16/row)
                    tmp = prpool.tile([128, R * D], bf16, name="tmp", tag="tmp")
                    nc.vector.tensor_mul(tmp[:], xs, bwt[:, 0:R * D])
                    t3 = tmp[:].rearrange("p (r d) -> p r d", r=R)
                    hb = hvpool.tile(
                        [128, R * (64 + 32 + 16)], bf16, name="hb", tag="hb"
                    )
                    h13 = hb[:, 0:R * 64].rearrange("p (r d) -> p r d", r=R)
                    h23 = hb[:, R * 64:R * 96].rearrange("p (r d) -> p r d", r=R)
                    h33 = hb[:, R * 96:R * 112].rearrange("p (r d) -> p r d", r=R)
                    nc.vector.tensor_add(h13, t3[:, :, 0:64], t3[:, :, 64:128])
                    nc.vector.tensor_add(h23, h13[:, :, 0:32], h13[:, :, 32:64])
                    nc.vector.tensor_add(h33, h23[:, :, 0:16], h23[:, :, 16:32])
                    tv = spool.tile([128, R], bf16, name="tv", tag="tv")
                    nc.vector.reduce_sum(tv[:], h33, axis=mybir.AxisListType.X)

                    ev = spool.tile([128, R], bf16, name="ev", tag="ev")
                    nc.scalar.activation(
                        ev[:], tv[:], mybir.ActivationFunctionType.Exp,
                        accum_out=den_all[:, t:t + 1],
                    )
                    # 4-row batched num matmuls: psum[4,512] accumulates
                    # ev[:,4g:4g+4].T @ X[:, 4 blocks]; diag extracted on host
                    for g in range(R // 4):
                        nc.tensor.matmul(
                            num_ps[:],
                            ev[:, 4 * g:4 * g + 4],
                            xs[:, 4 * g * D:(4 * g + 4) * D],
                            start=(i == 0),
                            stop=(i == NMM - 1),
                        )
                        i += 1

            # den only depends on the exps — finishes during the last matmuls
            den_vec = accpool.tile([128, 1], f32)
            nc.vector.reduce_sum(
                den_vec[:], den_all[:], axis=mybir.AxisListType.X
            )
            nc.sync.dma_start(out_den[:, :], den_vec[:])

            # PSUM evacuation on the Scalar engine (closer to PSUM, and runs
            # in parallel with the DVE den reduce)
            num_sb = accpool.tile([4, 512], f32)
            nc.scalar.copy(num_sb[:], num_ps[:])
            nc.sync.dma_start(out_num[:, :], num_sb[:])

    nc.compile()
    return nc


def kernel(X, W_sum, W_alpha, a_alpha, b_alpha):
    global _nc_cache, LAST_RESULTS
    import ml_dtypes
    from concourse.bass_utils import run_bass_kernel_spmd

    if _nc_cache is None:
        _nc_cache = _build()
    nc = _nc_cache

    X = np.ascontiguousarray(np.asarray(X), dtype=np.float32)
    W_sum = np.asarray(W_sum, dtype=np.float32)
    W_alpha = np.asarray(W_alpha, dtype=np.float32)
    b_alpha = np.asarray(b_alpha, dtype=np.float32)

    b_w = (b_alpha[:, 0] @ W_alpha).astype(np.float32)
    B = np.ascontiguousarray(
        np.tile(b_w[None, :], (128, 1)).astype(ml_dtypes.bfloat16)
    )

    Xp = np.zeros((NCORES * CAP, D), dtype=np.float32)
    Xp[:N] = X
    shards = Xp.reshape(NCORES, CAP, D)
    Z = np.zeros((8, R_MAX * D), dtype=ml_dtypes.bfloat16)
    in_maps = [
        {"x": np.ascontiguousarray(shards[c]), "bw": B, "zt": Z}
        for c in range(NCORES)
    ]

    res = run_bass_kernel_spmd(nc, in_maps, core_ids=list(range(NCORES)))
    LAST_RESULTS = res

    num = np.zeros(D, dtype=np.float64)
    den = 0.0
    for r in res.results:
        on = r["out_num"].astype(np.float64)  # [4, 512], diag blocks valid
        for m in range(4):
            num += on[m, m * D:(m + 1) * D]
        den += float(r["out_den"][:, 0].astype(np.float64).sum())
    den -= float(EXP0)  # zero slots/pad rows each contribute exp(0) = 1

    sum_output = (num / den).astype(np.float32)
    return (sum_output @ W_sum).astype(np.float32)


# revision 20
# speedup vs baseline: 1.0929x; 1.0929x over previous
"""Trainium2 Bass kernel for nn_AttentionModule (GNN attention pooling).

Math (reference):
    a_w = a_alpha[:,0] @ W_alpha ; b_w = b_alpha[:,0] @ W_alpha
    alpha_j = exp(a_w @ X[0] + X_j @ b_w)
    out = ((alpha @ X) / jnp.sum(alpha)) @ W_sum

Since the output is a ratio, the constant factor exp(a_w @ X[0]) cancels
exactly, so each device only needs one streaming pass over its shard of X:
    t_j = X_j . b_w ; e_j = exp(t_j)
    num = sum_j e_j * X_j   (D floats)   den = sum_j e_j   (1 float)
Host: reduce the 8 partials, divide, project through W_sum (tiny).

Sharding: X row-wise across 8 cores. Rows landing in zero-filled slots
contribute exp(0)=1 to den (subtracted exactly on the host) and 0 to num.

Datapath: X streams from HBM as f32 (full memory traffic), cast to bf16
during the DMA (SWDGE inline cast). Measured: SDMA engine 15 runs ~25%
slower than the rest on SWDGE traffic (descriptor-ring port contention)
and partition p maps to engine p%16, so uniform tiles leave every tile's
completion semaphore trailing engine 15 by up to 8 us. Fix: on the big
steady tiles, partitions p%16==15 carry S<R rows (S/R ~ 0.70), delivered
by single-strided-partition-dim DMAs (multi-dim partition APs miscompile
on the SWDGE cast path; a 5-way stride decomposition is exact). Edge
tiles are small and stay uniform (1 DMA each) since Q7 descriptor
generation costs ~0.63 us per dma_start. Every tile has a unique SBUF
buffer (whole bf16 shard ~6.4 MB) so the stream never stalls on compute.

num accumulates via 4-row batched matmuls (psum[4,512] +=
ev[:,4g:4g+4].T @ X[:, 4 blocks]); the host extracts the block-diagonal.
All accumulations (t dot products, den, PSUM num) stay in f32.
"""

import numpy as np

D = 128
NCORES = 8
N = 200000
# tiles: (R rows/partition, S rows on slow partitions; S == R -> uniform)
# First tiles are uniform and sized so the stream stays just ahead of the
# (saturated) DVE — early DVE idle lands 1:1 at the end of the kernel.
TILES = [(12, 12), (16, 16), (28, 20), (28, 20), (28, 20), (28, 20),
         (28, 20), (28, 20), (4, 4)]
T = len(TILES)
R_MAX = max(r for r, _ in TILES)
CAP = sum(120 * r + 8 * s for r, s in TILES)
ZSLOTS = sum(8 * (r - s) for r, s in TILES)
EXP0 = NCORES * ZSLOTS + (NCORES * CAP - N)  # total exp(0) contributions

_nc_cache = None
LAST_RESULTS = None


def _build():
    import concourse.bacc as bacc
    import concourse.bass as bass
    import concourse.mybir as mybir
    import concourse.tile as tile

    f32 = mybir.dt.float32
    bf16 = mybir.dt.bfloat16
    nc = bacc.Bacc("TRN2", target_bir_lowering=False, debug=False)

    NMM = sum(r // 4 for r, _ in TILES)   # 4-row batched matmuls

    x = nc.dram_tensor("x", [CAP, D], f32, kind="ExternalInput")
    bw = nc.dram_tensor("bw", [128, D], bf16, kind="ExternalInput")
    zt = nc.dram_tensor("zt", [8, R_MAX * D], bf16, kind="ExternalInput")
    out_num = nc.dram_tensor("out_num", [4, 512], f32, kind="ExternalOutput")
    out_den = nc.dram_tensor("out_den", [128, 1], f32, kind="ExternalOutput")

    with tile.TileContext(nc, pool_alloc_mode="queue") as tc:
        with (
            tc.tile_pool(name="xb", bufs=1) as xbpool,
            tc.tile_pool(name="pr", bufs=2) as prpool,
            tc.tile_pool(name="hv", bufs=2) as hvpool,
            tc.tile_pool(name="sm", bufs=3) as spool,
            tc.tile_pool(name="acc", bufs=1) as accpool,
            tc.tile_pool(name="ps", bufs=1, space=bass.MemorySpace.PSUM) as pspool,
            tc.tile_pool(name="pw", bufs=1, space=bass.MemorySpace.PSUM) as pwpool,
        ):
            bsmall = accpool.tile([128, D], bf16)
            nc.sync.dma_start(bsmall[:], bw[:, :])
            # replicate b_w R_MAX times along the free dim (one-time)
            bwt = accpool.tile([128, R_MAX * D], bf16)
            nc.vector.tensor_copy(
                bwt[:].rearrange("p (r d) -> p r d", r=R_MAX),
                bsmall[:].rearrange("p (u d) -> p u d", u=1).broadcast_to(
                    [128, R_MAX, D]
                ),
            )

            den_all = accpool.tile([128, T], f32)
            num_ps = pspool.tile([4, 512], f32, name="num_ps", tag="ps")

            # unique bf16 buffer per tile + one-time zero fill of the slow
            # partitions' tail slots on ragged tiles (rows S..R never
            # DMA-written)
            xts = []
            for t, (R, S) in enumerate(TILES):
                xt = xbpool.tile([128, R * D], bf16, name=f"xt{t}", tag=f"xt{t}")
                if S < R:
                    nc.sync.dma_start(
                        xt[:].rearrange("(g j) f -> g j f", j=16)[
                            :, 15, S * D:R * D],
                        zt.ap()[:, 0:(R - S) * D],
                    )
                xts.append(xt)

            with nc.allow_low_precision("t stats kept in bf16; exp reads them"):
                row0 = 0
                i = 0
                for t, (R, S) in enumerate(TILES):
                    xt = xts[t]
                    if S == R:
                        # uniform tile: single SWDGE cast DMA, all partitions
                        nc.gpsimd.dma_start(
                            xt[:],
                            x.ap()[row0:row0 + 128 * R, :].rearrange(
                                "(p r) d -> p (r d)", p=128
                            ).opt(),
                        )
                        row0 += 128 * R
                    else:
                        # ragged tile: five single-strided-partition DMAs;
                        # p%16==15 (engine 15) carries S < R rows
                        for start, step, n, rr in [
                            (0, 2, 64, R), (1, 4, 32, R), (3, 8, 16, R),
                            (7, 16, 8, R), (15, 16, 8, S),
                        ]:
                            nc.gpsimd.dma_start(
                                xt[:].rearrange(
                                    "(g j) f -> g j f", j=step
                                )[:, start, 0:rr * D],
                                x.ap()[row0:row0 + n * rr, :].rearrange(
                                    "(g r) d -> g (r d)", g=n
                                ),
                            )
                            row0 += n * rr
                    xs = xt[:, 0:R * D]

                    # t_j = X_j . b_w : multiply at DVE 2x, shrink with
                    # 2x halving adds, then the 1x-capped reduce (16/row)
                    tmp = prpool.tile([128, R * D], bf16, name="tmp", tag="tmp")
                    nc.vector.tensor_mul(tmp[:], xs, bwt[:, 0:R * D])
                    t3 = tmp[:].rearrange("p (r d) -> p r d", r=R)
                    hb = hvpool.tile(
                        [128, R * (64 + 32 + 16)], bf16, name="hb", tag="hb"
                    )
                    h13 = hb[:, 0:R * 64].rearrange("p (r d) -> p r d", r=R)
                    h23 = hb[:, R * 64:R * 96].rearrange("p (r d) -> p r d", r=R)
                    h33 = hb[:, R * 96:R * 112].rearrange("p (r d) -> p r d", r=R)
                    nc.vector.tensor_add(h13, t3[:, :, 0:64], t3[:, :, 64:128])
                    nc.vector.tensor_add(h23, h13[:, :, 0:32], h13[:, :, 32:64])
                    nc.vector.tensor_add(h33, h23[:, :, 0:16], h23[:, :, 16:32])
                    tv = spool.tile([128, R], bf16, name="tv", tag="tv")
                    nc.vector.reduce_sum(tv[:], h33, axis=mybir.AxisListType.X)

                    ev = spool.tile([128, R], bf16, name="ev", tag="ev")
                    nc.scalar.activation(
                        ev[:], tv[:], mybir.ActivationFunctionType.Exp,
                        accum_out=den_all[:, t:t + 1],
                    )
                    # 4-row batched num matmuls: psum[4,512] accumulates
                    # ev[:,4g:4g+4].T @ X[:, 4 blocks]; diag extracted on host
                    for g in range(R // 4):
                        nc.tensor.matmul(
                            num_ps[:],
                            ev[:, 4 * g:4 * g + 4],
                            xs[:, 4 * g * D:(4 * g + 4) * D],
                            start=(i == 0),
                            stop=(i == NMM - 1),
                        )
                        i += 1

            # den only depends on the exps — finishes during the last matmuls
            den_vec = accpool.tile([128, 1], f32)
            nc.vector.reduce_sum(
                den_vec[:], den_all[:], axis=mybir.AxisListType.X
            )
            nc.sync.dma_start(out_den[:, :], den_vec[:])

            # PSUM evacuation on the Scalar engine (closer to PSUM, and runs
            # in parallel with the DVE den reduce)
            num_sb = accpool.tile([4, 512], f32)
            nc.scalar.copy(num_sb[:], num_ps[:])
            nc.sync.dma_start(out_num[:, :], num_sb[:])

    nc.compile()
    return nc


def kernel(X, W_sum, W_alpha, a_alpha, b_alpha):
    global _nc_cache, LAST_RESULTS
    import ml_dtypes
    from concourse.bass_utils import run_bass_kernel_spmd

    if _nc_cache is None:
        _nc_cache = _build()
    nc = _nc_cache

    X = np.ascontiguousarray(np.asarray(X), dtype=np.float32)
    W_sum = np.asarray(W_sum, dtype=np.float32)
    W_alpha = np.asarray(W_alpha, dtype=np.float32)
    b_alpha = np.asarray(b_alpha, dtype=np.float32)

    b_w = (b_alpha[:, 0] @ W_alpha).astype(np.float32)
    B = np.ascontiguousarray(
        np.tile(b_w[None, :], (128, 1)).astype(ml_dtypes.bfloat16)
    )

    Xp = np.zeros((NCORES * CAP, D), dtype=np.float32)
    Xp[:N] = X
    shards = Xp.reshape(NCORES, CAP, D)
    Z = np.zeros((8, R_MAX * D), dtype=ml_dtypes.bfloat16)
    in_maps = [
        {"x": np.ascontiguousarray(shards[c]), "bw": B, "zt": Z}
        for c in range(NCORES)
    ]

    res = run_bass_kernel_spmd(nc, in_maps, core_ids=list(range(NCORES)))
    LAST_RESULTS = res

    num = np.zeros(D, dtype=np.float64)
    den = 0.0
    for r in res.results:
        on = r["out_num"].astype(np.float64)  # [4, 512], diag blocks valid
        for m in range(4):
            num += on[m, m * D:(m + 1) * D]
        den += float(r["out_den"][:, 0].astype(np.float64).sum())
    den -= float(EXP0)  # zero slots/pad rows each contribute exp(0) = 1

    sum_output = (num / den).astype(np.float32)
    return (sum_output @ W_sum).astype(np.float32)
